# revision 44
# baseline (speedup 1.0000x reference)
"""Trainium2 Bass kernel for the CN coupling-block problem (nn_CN_69312182223156).

Math (per subnet s on half-features x_s with conditioner c):
    h   = relu(c @ W1 + b1)                       # [B, 50]
    p   = h @ W2 + b2                             # [B, 9696]
    m1, b1p, m2 = p[:, :3200], p[:, 3200:6400], p[:, 6400:9600]   (viewed [B,32,100])
    bias2, eps, alpha = p[:, 9600:9632], p[:, 9632:9664]/10, p[:, 9664:]/10
    z   = x*m1 + b1p
    num = sum_l elu(z)*m2 ;  den = sum_l relu(-m1*m2) + 1
    y   = exp(alpha) * (x + 0.8*sigmoid(eps)*num/den) + bias2

Subnet 1: x=x1, c=x2.  Subnet 2: x=x2, c=y1.  Output concat([y1, y2]).

Strategy: pure data-parallel over 8 cores (2048 rows each), weights replicated.
Batch rows on SBUF partitions (tiles of 128). Biases folded into augmented
weights; b1 region carries b1+1 so z1 = z+1 and elu(z)+1 = max(z1, exp(z1-1))
with exp argument clamped via zn = min(z1, 1) (Schraudolph f16 bit-trick for
the exp).

den path: sum_l relu(-m1*m2) = -sum_l min(v, 0) with v = m1*m2, computed as
8 per-dim DVE tensor_scalar(min, add) ops whose free accum_out produces the
per-dim f32 sums directly (no materialized relu tensor, no fold for den).
num path: t = w*m2 folded by an in-place halving tree over big.
Tail uses 0.8*sigmoid(eps/10) = 0.4*(1+tanh(eps/20)) so Act stays in the
exp table-set.

Engine split (tuned against TimelineSim): Act does the three PSUM->SBUF f16
copies (consumer-aligned order m2s, m1s, [2 zm dims], b1s) + transcendentals;
Pool does v (tt-mult) and e2 (bit-trick tensor_scalar); DVE does everything
else (zm, z1, zn, w, t, den-accums, fold, tail arithmetic); PE does all
matmuls/transposes.

Emission is software-pipelined over a flat stream of 256 chunk-units:
slot k emits matmuls/copies/zm/z1/v of chunk k, the mid stage (zn/e2/den
accums) of chunk k-1, and the late stage (w, t) of chunk k-2, so the
in-order engine queues never stall on the cross-engine z1->zn->e2->w chain.
Fold+tail of a (subnet,tile) unit is emitted during the next unit's stream.
"""

import numpy as np

B = 16384
DIM = 32
LS = 100
NCORES = 8
BC = B // NCORES          # rows per core
NT = BC // 128            # 128-row tiles per core
DL = DIM * LS             # 3200
PW = 3 * DL + 3 * DIM     # 9696 params per row
CHUNK = 800               # params per elementwise chunk (8 dims x 100)
HALF = 400                # params per PSUM-bank matmul
NCHUNK = DL // CHUNK      # 4
DPC = CHUNK // LS         # 8 dims per chunk

# ---- engine-balance knobs (tuned against TimelineSim) ----
E_BIT_A = 1477.3196
E_BIT_B = 13823.9  # 15360 - A - 59.3 (Schraudolph corr) + 0.5 (trunc comp)
# per-op engine assignment: "D" = DVE (vector), "A" = Act (scalar),
# "P" = Pool (gpsimd). For per-dim zm: counts per engine (sum = 8).
ZM_D, ZM_A, ZM_P = 6, 2, 0
COPIES = {"m1s": "A", "b1s": "A", "m2s": "A"}
ZN_ENG = "D"
E2_ENG = "P"
W_ENG = "D"
T_ENG = "D"
V_ENG = "P"               # v = zm*m2 (replaces u = m1*m2)
Z1_ENG = "D"
SKEW = 2                  # chunk-slots of software-pipeline skew for w/t
RSKEW = 1                 # chunk-slots of skew for r
FOLD_INPLACE = True       # fold tree writes back into big (saves SBUF)
EW_BUFS = 6
PMM_BUFS = 3
HT_ENG = "A"              # hT relu: A or D
CONDT_ENG = "A"           # condT copy: A or D
SX_ENG = "D"
YP_ENG = "D"
BIG_BUFS = 3
TAIL_BUFS = 3
MID_BUFS = 4
HEAD_PREFETCH_AT = 4   # chunk index at which next unit's head is emitted (4=off)

_cache = {}


def _build_program():
    import concourse.bass as bass
    import concourse.tile as tile
    import concourse.mybir as mybir
    from concourse import bacc, masks

    f32 = mybir.dt.float32
    f16 = mybir.dt.float16
    Alu = mybir.AluOpType
    Act = mybir.ActivationFunctionType

    nc = bacc.Bacc("TRN2", target_bir_lowering=False)

    x_d = nc.dram_tensor("x", [BC, 2 * DIM], f32, kind="ExternalInput")
    w1a = [nc.dram_tensor(f"w1a{s}", [DIM + 1, 51], f16, kind="ExternalInput")
           for s in (1, 2)]
    w2a = [nc.dram_tensor(f"w2a{s}", [51, PW + DIM], f16, kind="ExternalInput")
           for s in (1, 2)]
    y_d = nc.dram_tensor("y", [BC, 2 * DIM], f32, kind="ExternalOutput")

    with tile.TileContext(nc) as tc:
        with (
            tc.tile_pool(name="const", bufs=1) as const,
            tc.tile_pool(name="per", bufs=1) as per,
            tc.tile_pool(name="mid", bufs=MID_BUFS) as mid,
            tc.tile_pool(name="ew", bufs=EW_BUFS) as ew,
            tc.tile_pool(name="big", bufs=BIG_BUFS) as bigp,
            tc.tile_pool(name="fold", bufs=2) as foldp,
            tc.tile_pool(name="tail", bufs=TAIL_BUFS) as tailp,
            tc.tile_pool(name="pmm", bufs=PMM_BUFS, space="PSUM") as pmm,
            tc.tile_pool(name="psm", bufs=2, space="PSUM") as psm,
        ):
            # ---- constants (DMA order tuned for pipeline rampup: the
            # first tile's conditioner (xf0) and the first chunk's weight
            # columns must land before the rest of the ~1MB weight loads,
            # otherwise every engine idles ~14us at program start) ----
            w1s = []
            w2s = []
            xfs, youts = {}, {}
            identf = const.tile([128, 128], f32, tag="identf")
            masks.make_identity(nc, identf[:])

            def load_xf(it):
                r0 = it * 128
                xf = per.tile([128, 2 * DIM + 1], f32, tag=f"xf{it}",
                              name=f"xf{it}")
                nc.sync.dma_start(xf[:, 0:2 * DIM], x_d[r0:r0 + 128, :])
                nc.gpsimd.memset(xf[:, 2 * DIM:], 1.0)
                xfs[it] = xf
                youts[it] = per.tile([128, 2 * DIM], f32, tag=f"y_out{it}",
                                     name=f"y_out{it}")

            load_xf(0)
            t1 = const.tile([DIM + 1, 51], f16, tag="w1_0")
            nc.sync.dma_start(t1, w1a[0][:])
            w1s.append(t1)
            # subnet-1 weights, split so chunk-0 columns arrive first
            t2 = const.tile([51, PW + DIM], f16, tag="w2_0")
            PIECE = 3 * CHUNK
            for o in range(0, PW + DIM, PIECE):
                hi = min(o + PIECE, PW + DIM)
                nc.sync.dma_start(t2[:, o:hi], w2a[0][:, o:hi])
            w2s.append(t2)
            for it in range(1, 4):
                load_xf(it)
            t1b = const.tile([DIM + 1, 51], f16, tag="w1_1")
            nc.sync.dma_start(t1b, w1a[1][:])
            w1s.append(t1b)
            t2b = const.tile([51, PW + DIM], f16, tag="w2_1")
            nc.sync.dma_start(t2b, w2a[1][:])
            w2s.append(t2b)
            for it in range(4, NT):
                load_xf(it)

            def unit_head(s, it):
                """Conditioner transpose + hT for one (subnet, tile) unit."""
                xf = xfs[it]
                if s == 0:
                    # conditioner for subnet 1: [x2 | 1]^T  -> [33, 128]
                    ct_ps = psm.tile([DIM + 1, 128], f32, tag="tp")
                    nc.tensor.transpose(ct_ps, xf[:, DIM:2 * DIM + 1], identf)
                    condT = mid.tile([DIM + 1, 128], f16, tag="condT")
                    if CONDT_ENG == "A":
                        nc.scalar.copy(condT, ct_ps)
                    else:
                        nc.vector.tensor_copy(condT, ct_ps)
                else:
                    # conditioner for subnet 2: [y1 | 1]^T
                    y_out = youts[it]
                    c2_ps = psm.tile([DIM, 128], f32, tag="tp")
                    nc.tensor.transpose(c2_ps, y_out[:, 0:DIM], identf)
                    condT = mid.tile([DIM + 1, 128], f16, tag="condT2")
                    if CONDT_ENG == "A":
                        nc.scalar.copy(condT[0:DIM, :], c2_ps)
                    else:
                        nc.vector.tensor_copy(condT[0:DIM, :], c2_ps)
                    nc.gpsimd.memset(condT[DIM:DIM + 1, :], 1.0)
                h_ps = psm.tile([51, 128], f32, tag="tp")
                nc.tensor.matmul(h_ps, w1s[s], condT, start=True, stop=True)
                hT = mid.tile([51, 128], f16, tag="hT")
                if HT_ENG == "A":
                    nc.scalar.activation(hT, h_ps, Act.Relu)
                else:
                    nc.vector.tensor_scalar(hT, h_ps, 0.0, None, Alu.max)
                big = bigp.tile([128, DIM, LS], f16, tag="big")
                minacc = tailp.tile([128, DIM], f32, tag="minacc")
                return {"hT": hT, "big": big, "minacc": minacc}

            def chunk_early(s, it, c, st):
                """PE matmuls, PSUM->SBUF copies, zm/z1/u, zn/e2/r."""
                hT, big = st["hT"], st["big"]
                xf = xfs[it]
                base = c * 3 * CHUNK
                xc32 = xf[:, s * DIM:(s + 1) * DIM]

                def copy_eng(eng, dst, src):
                    if eng == "A":
                        nc.scalar.copy(dst, src)
                    elif eng == "P":
                        nc.gpsimd.tensor_copy(dst, src)
                    else:
                        nc.vector.tensor_copy(dst, src)

                def ts_eng(eng, dst, src, s1, s2, op0, op1):
                    e = nc.gpsimd if eng == "P" else nc.vector
                    if s2 is None:
                        e.tensor_scalar(dst, src, s1, None, op0)
                    else:
                        e.tensor_scalar(dst, src, s1, s2, op0, op1)

                def tt_eng(eng, dst, a, b, op):
                    e = nc.gpsimd if eng == "P" else nc.vector
                    e.tensor_tensor(dst, a, b, op)

                # pair tiles for the late stage: z1/e2/m2s halves are
                # written per-chunk; w and t then run as single [128,1600]
                # ops per chunk-pair (amortizes the DVE per-op init).
                pc, pi = c % 2, c // 2
                if pc == 0:
                    z1P = ew.tile([128, 2, CHUNK], f16, tag="z1")
                    m2sP = ew.tile([128, 2, CHUNK], f16, tag="m2s")
                    e2P = ew.tile([128, 2, CHUNK], mybir.dt.int16, tag="e2")
                    st[("pair", pi)] = (z1P, m2sP, e2P)
                z1P, m2sP, e2P = st[("pair", pi)]

                # consumer-aligned copy order: m2s+m1s unblock u/zm after
                # two Act ops; b1s (only needed by z1) last.
                m2p = pmm.tile([128, 2, 512], f32, tag="mm")
                for hh in range(2):
                    o = base + 2 * CHUNK + hh * HALF
                    nc.tensor.matmul(m2p[:, hh, 0:HALF], hT,
                                     w2s[s][:, o:o + HALF],
                                     start=True, stop=True)
                m2s = m2sP[:, pc, :]
                m2s2 = m2s.rearrange("p (h q) -> p h q", h=2)
                copy_eng(COPIES["m2s"], m2s2, m2p[:, :, 0:HALF])

                m1p = pmm.tile([128, 2, 512], f32, tag="mm")
                for hh in range(2):
                    o = base + hh * HALF
                    nc.tensor.matmul(m1p[:, hh, 0:HALF], hT,
                                     w2s[s][:, o:o + HALF],
                                     start=True, stop=True)
                m1s = ew.tile([128, CHUNK], f16, tag="m1s")
                m1s2 = m1s.rearrange("p (h q) -> p h q", h=2)
                copy_eng(COPIES["m1s"], m1s2, m1p[:, :, 0:HALF])

                # -- z = x*m1 (per-dim broadcast), split across engines --
                # (emitted before b1s so Act's zm dims precede b1s: z1's
                # last dependency is then Act's final op of this chunk)
                zm = ew.tile([128, CHUNK], f16, tag="zm")
                zm3 = zm.rearrange("p (d l) -> p d l", l=LS)
                m1s3 = m1s.rearrange("p (d l) -> p d l", l=LS)
                for j in range(DPC):
                    xj = xc32[:, c * DPC + j:c * DPC + j + 1]
                    if j < ZM_D:
                        nc.vector.tensor_scalar_mul(
                            zm3[:, j, :], m1s3[:, j, :], xj)
                    elif j < ZM_D + ZM_A:
                        nc.scalar.activation(zm3[:, j, :], m1s3[:, j, :],
                                             Act.Copy, scale=xj)
                    else:
                        nc.gpsimd.tensor_scalar_mul(
                            zm3[:, j, :], m1s3[:, j, :], xj)

                b1p = pmm.tile([128, 2, 512], f32, tag="mm")
                for hh in range(2):
                    o = base + CHUNK + hh * HALF
                    nc.tensor.matmul(b1p[:, hh, 0:HALF], hT,
                                     w2s[s][:, o:o + HALF],
                                     start=True, stop=True)
                b1s = ew.tile([128, CHUNK], f16, tag="b1s")
                b1s2 = b1s.rearrange("p (h q) -> p h q", h=2)
                copy_eng(COPIES["b1s"], b1s2, b1p[:, :, 0:HALF])
                # z1 = zm + (b1+1) into its pair-tile half
                z1 = z1P[:, pc, :]
                z1e = Z1_ENG[c % len(Z1_ENG)] if isinstance(Z1_ENG, list) else Z1_ENG
                tt_eng(z1e, z1, zm, b1s, Alu.add)
                # v = m1*m2 (den path; needs only the first two Act copies)
                v = ew.tile([128, CHUNK], f16, tag="v")
                tt_eng(V_ENG, v, m1s, m2s, Alu.mult)
                st[("late", c)] = [z1, None, m2s, v, zm]

            def chunk_r(s, it, c, st):
                """Mid stage (slot+1): zn/e2 (exp bit-trick) and den partial
                sums: per-dim min(m1*m2, 0) accumulated into minacc via
                tensor_scalar accum_out (free); den = 1 - sum since
                sum_l relu(-v) = -sum_l min(v, 0).
                Scratch writes go into the dead zm tile regions."""
                minacc = st["minacc"]
                # den partials for this chunk (needs only v of this chunk)
                late = st[("late", c)]
                z1, _, m2s, v, zm = late
                v3 = v.rearrange("p (d l) -> p d l", l=LS)
                zm3 = zm.rearrange("p (d l) -> p d l", l=LS)
                for j in range(DPC):
                    d = c * DPC + j
                    nc.vector.tensor_scalar(
                        zm3[:, j, :], v3[:, j, :], 0.0, 0.0,
                        Alu.min, Alu.add, accum_out=minacc[:, d:d + 1])
                if c % 2 == 0:
                    return
                # zn/e2 pair-granular over [128,1600]:
                # zn = min(z1, 1); e = exp(zn-1) via f16 bit-trick:
                # int16(A*zn + B) reinterpreted as f16 (Schraudolph).
                pi = c // 2
                z1P, m2sP, e2P = st[("pair", pi)]
                zn = ew.tile([128, 2, CHUNK], f16, tag="zn")
                ts_eng_g(ZN_ENG, zn, z1P, 1.0, None, Alu.min, Alu.bypass)
                ts_eng_g(E2_ENG, e2P, zn, E_BIT_A, E_BIT_B, Alu.mult, Alu.add)

            def ts_eng_g(eng, dst, src, s1, s2, op0, op1):
                e = nc.gpsimd if eng == "P" else nc.vector
                if s2 is None:
                    e.tensor_scalar(dst, src, s1, None, op0)
                else:
                    e.tensor_scalar(dst, src, s1, s2, op0, op1)

            def chunk_late(s, it, c, st):
                """w = max(z1, e); t = w*m2 into big — pair-granular
                ([128,1600] ops, emitted at odd chunk indices)."""
                if c % 2 == 0:
                    return
                big = st["big"]
                pi = c // 2
                z1P, m2sP, e2P = st.pop(("pair", pi))
                st.pop(("late", c - 1))
                st.pop(("late", c))
                w = ew.tile([128, 2, CHUNK], f16, tag="w")
                e = nc.gpsimd if W_ENG == "P" else nc.vector
                e.tensor_tensor(w, z1P, e2P[:, :, :].bitcast(f16), Alu.max)
                tdst = big[:, 2 * pi * DPC:(2 * pi + 2) * DPC, :]
                e = nc.gpsimd if T_ENG == "P" else nc.vector
                e.tensor_tensor(tdst,
                                w.rearrange("p h (d l) -> p (h d) l", l=LS),
                                m2sP.rearrange("p h (d l) -> p (h d) l", l=LS),
                                Alu.mult)

            def fold_tail(s, it, st):
                hT, big = st["hT"], st["big"]
                xf = xfs[it]
                y_out = youts[it]
                xc32 = xf[:, s * DIM:(s + 1) * DIM]
                minacc = st["minacc"]
                # ---- fold tree over l: [128,32,100] -> [128,32] f32 ----
                n1 = big[:, :, 0:50]
                nc.vector.tensor_add(n1, big[:, :, 0:50], big[:, :, 50:100])
                n2 = big[:, :, 0:25]
                nc.vector.tensor_add(n2, n1[:, :, 0:25], n1[:, :, 25:50])
                n3 = big[:, :, 25:37]
                nc.vector.tensor_add(n3, n2[:, :, 0:12], n2[:, :, 12:24])
                n4 = big[:, :, 37:43]
                nc.vector.tensor_add(n4, n3[:, :, 0:6], n3[:, :, 6:12])
                n5 = big[:, :, 43:46]
                nc.vector.tensor_add(n5, n4[:, :, 0:3], n4[:, :, 3:6])
                s1 = big[:, :, 46]
                nc.vector.tensor_add(s1, n5[:, :, 0], n5[:, :, 1])
                s2 = big[:, :, 47]
                nc.vector.tensor_add(s2, n5[:, :, 2], n2[:, :, 24])
                numden = tailp.tile([128, DIM], f32, tag="numden")
                nc.vector.tensor_add(numden, s1, s2)

                # ---- tail (bias2 | eps | alpha | S2) ----
                tp = psm.tile([128, 4 * DIM], f32, tag="tp")
                nc.tensor.matmul(tp, hT, w2s[s][:, 3 * DL:3 * DL + 4 * DIM],
                                 start=True, stop=True)
                b2p = tp[:, 0:DIM]
                epp = tp[:, DIM:2 * DIM]
                alp = tp[:, 2 * DIM:3 * DIM]
                s2p = tp[:, 3 * DIM:4 * DIM]

                den = tailp.tile([128, DIM], f32, tag="den")
                nc.gpsimd.tensor_scalar(den, minacc, -1.0, 1.0, Alu.mult, Alu.add)
                rec = tailp.tile([128, DIM], f32, tag="rec")
                nc.vector.reciprocal_approx_fast(rec, den)
                # 0.8*sigmoid(eps/10) = 0.4*(1 + tanh(eps/20)); Tanh shares
                # the exp table-set so no ACT_TABLE_LOAD thrash.
                t2 = tailp.tile([128, DIM], f32, tag="t2")
                nc.scalar.activation(t2, epp, Act.Tanh, scale=0.05)
                ea = tailp.tile([128, DIM], f32, tag="ea")
                nc.scalar.activation(ea, alp, Act.Exp, scale=0.1)
                nums = tailp.tile([128, DIM], f32, tag="nums")
                nc.vector.tensor_sub(nums, numden, s2p)
                frac = tailp.tile([128, DIM], f32, tag="frac")
                nc.vector.scalar_tensor_tensor(
                    frac, in0=nums, scalar=0.4, in1=rec, op0=Alu.mult, op1=Alu.mult)
                q = tailp.tile([128, DIM], f32, tag="q")
                nc.vector.scalar_tensor_tensor(
                    q, in0=t2, scalar=1.0, in1=frac, op0=Alu.add, op1=Alu.mult)
                sx = tailp.tile([128, DIM], f32, tag="sx")
                e = nc.gpsimd if SX_ENG == "P" else nc.vector
                e.tensor_tensor(sx, q, xc32, Alu.add)
                yp = tailp.tile([128, DIM], f32, tag="yp")
                e = nc.gpsimd if YP_ENG == "P" else nc.vector
                e.tensor_tensor(yp, ea, sx, Alu.mult)
                nc.vector.tensor_add(y_out[:, s * DIM:(s + 1) * DIM], yp, b2p)
                if s == 1:
                    r0 = it * 128
                    nc.sync.dma_start(y_d[r0:r0 + 128, :], y_out)

            # ---- flat software-pipelined emission ----
            units = [(0, it) for it in range(NT)] + [(1, it) for it in range(NT)]
            slots = [(ui, c) for ui in range(len(units)) for c in range(NCHUNK)]
            NSL = len(slots)
            states = {}
            pending_ft = []  # unit indices whose fold/tail is not yet emitted
            for k in range(NSL + SKEW):
                if k < NSL:
                    ui, c = slots[k]
                    s, it = units[ui]
                    if c == 0 and ui not in states:
                        states[ui] = unit_head(s, it)
                    chunk_early(s, it, c, states[ui])
                    if c == HEAD_PREFETCH_AT and ui + 1 < len(units):
                        ns_, nit = units[ui + 1]
                        states[ui + 1] = unit_head(ns_, nit)
                ri = k - RSKEW
                if 0 <= ri < NSL:
                    rui, rc = slots[ri]
                    rs, rit = units[rui]
                    chunk_r(rs, rit, rc, states[rui])
                li = k - SKEW
                if 0 <= li < NSL:
                    pui, pc = slots[li]
                    ps, pit = units[pui]
                    chunk_late(ps, pit, pc, states[pui])
                    if pc == NCHUNK - 1:
                        pending_ft.append(pui)
                if pending_ft and (k >= NSL or slots[k][1] == 1):
                    fui = pending_ft.pop(0)
                    fs, fit = units[fui]
                    fold_tail(fs, fit, states[fui])
                    del states[fui]
            for fui in pending_ft:
                fs, fit = units[fui]
                fold_tail(fs, fit, states[fui])
                del states[fui]

    nc.compile()
    return nc


def _prep_weights(W1, b1, W2, b2):
    w1a = np.concatenate([W1, b1[None, :]], axis=0).astype(np.float16)  # [33, 50]
    ones_col = np.zeros((DIM + 1, 1), dtype=np.float16)
    ones_col[DIM, 0] = 1.0
    w1a = np.concatenate([w1a, ones_col], axis=1)                       # [33, 51]
    w2a = np.concatenate([W2, b2[None, :]], axis=0)                     # [51, 9696] f32
    w2a = w2a.copy()
    w2a[50, DL:2 * DL] += 1.0   # bias1 region delivers b1+1 (z1 = z+1)
    # append S2 columns: S2[:, d] = sum_l w2a[:, mat2 region (d, l)]
    m2cols = w2a[:, 2 * DL:3 * DL].reshape(51, DIM, LS)
    s2 = m2cols.sum(axis=2)                                             # [51, DIM]
    w2a = np.concatenate([w2a, s2], axis=1)                             # [51, 9728]
    # interleave chunk-blocks: [m1_c | b1_c | m2_c] x 4 chunks, tails last
    out = np.empty_like(w2a)
    for c in range(NCHUNK):
        src_m1 = w2a[:, c * CHUNK:(c + 1) * CHUNK]
        src_b1 = w2a[:, DL + c * CHUNK:DL + (c + 1) * CHUNK]
        src_m2 = w2a[:, 2 * DL + c * CHUNK:2 * DL + (c + 1) * CHUNK]
        base = c * 3 * CHUNK
        out[:, base:base + CHUNK] = src_m1
        out[:, base + CHUNK:base + 2 * CHUNK] = src_b1
        out[:, base + 2 * CHUNK:base + 3 * CHUNK] = src_m2
    out[:, 3 * DL:] = w2a[:, 3 * DL:]
    return (np.ascontiguousarray(w1a),
            np.ascontiguousarray(out.astype(np.float16)))


def kernel(**inputs):
    from concourse.bass_utils import run_bass_kernel_spmd

    if "nc" not in _cache:
        _cache["nc"] = _build_program()
    nc = _cache["nc"]

    x = np.ascontiguousarray(inputs["x"], dtype=np.float32)
    w1a1, w2a1 = _prep_weights(inputs["s1_W1"], inputs["s1_b1"],
                               inputs["s1_W2"], inputs["s1_b2"])
    w1a2, w2a2 = _prep_weights(inputs["s2_W1"], inputs["s2_b1"],
                               inputs["s2_W2"], inputs["s2_b2"])

    in_maps = []
    for i in range(NCORES):
        in_maps.append({
            "x": x[i * BC:(i + 1) * BC],
            "w1a1": w1a1, "w2a1": w2a1,
            "w1a2": w1a2, "w2a2": w2a2,
        })

    last_err = None
    for attempt in range(3):
        try:
            res = run_bass_kernel_spmd(nc, in_maps, core_ids=list(range(NCORES)),
                                       **_cache.get("run_kwargs", {}))
            out = np.concatenate([r["y"] for r in res.results], axis=0)
            _cache["last_results"] = res
            return out
        except Exception as ex:  # transient NRT/device errors: retry
            last_err = ex
    raise last_err


# revision 45
# speedup vs baseline: 1.0647x; 1.0647x over previous
"""Trainium2 Bass kernel for the CN coupling-block problem (nn_CN_69312182223156).

Math (per subnet s on half-features x_s with conditioner c):
    h   = relu(c @ W1 + b1)                       # [B, 50]
    p   = h @ W2 + b2                             # [B, 9696]
    m1, b1p, m2 = p[:, :3200], p[:, 3200:6400], p[:, 6400:9600]   (viewed [B,32,100])
    bias2, eps, alpha = p[:, 9600:9632], p[:, 9632:9664]/10, p[:, 9664:]/10
    z   = x*m1 + b1p
    num = sum_l elu(z)*m2 ;  den = sum_l relu(-m1*m2) + 1
    y   = exp(alpha) * (x + 0.8*sigmoid(eps)*num/den) + bias2

Subnet 1: x=x1, c=x2.  Subnet 2: x=x2, c=y1.  Output concat([y1, y2]).

Strategy: pure data-parallel over 8 cores (2048 rows each), weights replicated.
Batch rows on SBUF partitions (tiles of 128). Biases folded into augmented
weights; b1 region carries b1+1 so z1 = z+1 and elu(z)+1 = max(z1, exp(z1-1))
with exp argument clamped via zn = min(z1, 1) (Schraudolph f16 bit-trick for
the exp).

den path: sum_l relu(-m1*m2) = -sum_l min(v, 0) with v = m1*m2, computed as
8 per-dim DVE tensor_scalar(min, add) ops whose free accum_out produces the
per-dim f32 sums directly (no materialized relu tensor, no fold for den).
num path: t = w*m2 folded by an in-place halving tree over big.
Tail uses 0.8*sigmoid(eps/10) = 0.4*(1+tanh(eps/20)) so Act stays in the
exp table-set.

Engine split (tuned against TimelineSim): Act does the three PSUM->SBUF f16
copies (consumer-aligned order m2s, m1s, [2 zm dims], b1s) + transcendentals;
Pool does v (tt-mult) and e2 (bit-trick tensor_scalar); DVE does everything
else (zm, z1, zn, w, t, den-accums, fold, tail arithmetic); PE does all
matmuls/transposes.

Emission is software-pipelined over a flat stream of 256 chunk-units:
slot k emits matmuls/copies/zm/z1/v of chunk k, the mid stage (zn/e2/den
accums) of chunk k-1, and the late stage (w, t) of chunk k-2, so the
in-order engine queues never stall on the cross-engine z1->zn->e2->w chain.
Fold+tail of a (subnet,tile) unit is emitted during the next unit's stream.
"""

import numpy as np

B = 16384
DIM = 32
LS = 100
NCORES = 8
BC = B // NCORES          # rows per core
NT = BC // 128            # 128-row tiles per core
DL = DIM * LS             # 3200
PW = 3 * DL + 3 * DIM     # 9696 params per row
CHUNK = 800               # params per elementwise chunk (8 dims x 100)
HALF = 400                # params per PSUM-bank matmul
NCHUNK = DL // CHUNK      # 4
DPC = CHUNK // LS         # 8 dims per chunk

# ---- engine-balance knobs (tuned against TimelineSim) ----
E_BIT_A = 1477.3196
E_BIT_B = 13823.9  # 15360 - A - 59.3 (Schraudolph corr) + 0.5 (trunc comp)
# per-op engine assignment: "D" = DVE (vector), "A" = Act (scalar),
# "P" = Pool (gpsimd). For per-dim zm: counts per engine (sum = 8).
ZM_D, ZM_A, ZM_P = 6, 2, 0
COPIES = {"m1s": "A", "b1s": "A", "m2s": "A"}
ZN_ENG = "D"
E2_ENG = "P"
W_ENG = "D"
T_ENG = "D"
V_ENG = "P"               # v = zm*m2 (replaces u = m1*m2)
Z1_ENG = "D"
SKEW = 2                  # chunk-slots of software-pipeline skew for w/t
RSKEW = 1                 # chunk-slots of skew for r
FOLD_INPLACE = True       # fold tree writes back into big (saves SBUF)
EW_BUFS = 6
PMM_BUFS = 3
HT_ENG = "A"              # hT relu: A or D
CONDT_ENG = "A"           # condT copy: A or D
SX_ENG = "D"
YP_ENG = "D"
BIG_BUFS = 3
TAIL_BUFS = 3
MID_BUFS = 4
HEAD_PREFETCH_AT = 4   # chunk index at which next unit's head is emitted (4=off)

_cache = {}


def _build_program():
    import concourse.bass as bass
    import concourse.tile as tile
    import concourse.mybir as mybir
    from concourse import bacc, masks

    f32 = mybir.dt.float32
    f16 = mybir.dt.float16
    Alu = mybir.AluOpType
    Act = mybir.ActivationFunctionType

    nc = bacc.Bacc("TRN2", target_bir_lowering=False)

    x_d = nc.dram_tensor("x", [BC, 2 * DIM], f32, kind="ExternalInput")
    w1a = [nc.dram_tensor(f"w1a{s}", [DIM + 1, 51], f16, kind="ExternalInput")
           for s in (1, 2)]
    w2a = [nc.dram_tensor(f"w2a{s}", [51, PW + DIM], f16, kind="ExternalInput")
           for s in (1, 2)]
    y_d = nc.dram_tensor("y", [BC, 2 * DIM], f32, kind="ExternalOutput")

    with tile.TileContext(nc) as tc:
        with (
            tc.tile_pool(name="const", bufs=1) as const,
            tc.tile_pool(name="per", bufs=1) as per,
            tc.tile_pool(name="mid", bufs=MID_BUFS) as mid,
            tc.tile_pool(name="ew", bufs=EW_BUFS) as ew,
            tc.tile_pool(name="big", bufs=BIG_BUFS) as bigp,
            tc.tile_pool(name="fold", bufs=2) as foldp,
            tc.tile_pool(name="tail", bufs=TAIL_BUFS) as tailp,
            tc.tile_pool(name="pmm", bufs=PMM_BUFS, space="PSUM") as pmm,
            tc.tile_pool(name="psm", bufs=2, space="PSUM") as psm,
        ):
            # ---- constants (DMA order tuned for pipeline rampup: the
            # first tile's conditioner (xf0) and the first chunk's weight
            # columns must land before the rest of the ~1MB weight loads,
            # otherwise every engine idles ~14us at program start) ----
            w1s = []
            w2s = []
            xfs, youts = {}, {}
            identf = const.tile([128, 128], f32, tag="identf")
            masks.make_identity(nc, identf[:])

            def load_xf(it):
                r0 = it * 128
                xf = per.tile([128, 2 * DIM + 1], f32, tag=f"xf{it}",
                              name=f"xf{it}")
                nc.sync.dma_start(xf[:, 0:2 * DIM], x_d[r0:r0 + 128, :])
                nc.gpsimd.memset(xf[:, 2 * DIM:], 1.0)
                xfs[it] = xf
                youts[it] = per.tile([128, 2 * DIM], f32, tag=f"y_out{it}",
                                     name=f"y_out{it}")

            load_xf(0)
            t1 = const.tile([DIM + 1, 51], f16, tag="w1_0")
            nc.sync.dma_start(t1, w1a[0][:])
            w1s.append(t1)
            # subnet-1 weights, split so chunk-0 columns arrive first
            t2 = const.tile([51, PW + DIM], f16, tag="w2_0")
            PIECE = 3 * CHUNK
            for o in range(0, PW + DIM, PIECE):
                hi = min(o + PIECE, PW + DIM)
                nc.sync.dma_start(t2[:, o:hi], w2a[0][:, o:hi])
            w2s.append(t2)
            for it in range(1, 4):
                load_xf(it)
            t1b = const.tile([DIM + 1, 51], f16, tag="w1_1")
            nc.sync.dma_start(t1b, w1a[1][:])
            w1s.append(t1b)
            t2b = const.tile([51, PW + DIM], f16, tag="w2_1")
            nc.sync.dma_start(t2b, w2a[1][:])
            w2s.append(t2b)
            for it in range(4, NT):
                load_xf(it)

            def unit_head(s, it):
                """Conditioner transpose + hT for one (subnet, tile) unit."""
                xf = xfs[it]
                if s == 0:
                    # conditioner for subnet 1: [x2 | 1]^T  -> [33, 128]
                    ct_ps = psm.tile([DIM + 1, 128], f32, tag="tp")
                    nc.tensor.transpose(ct_ps, xf[:, DIM:2 * DIM + 1], identf)
                    condT = mid.tile([DIM + 1, 128], f16, tag="condT")
                    if CONDT_ENG == "A":
                        nc.scalar.copy(condT, ct_ps)
                    else:
                        nc.vector.tensor_copy(condT, ct_ps)
                else:
                    # conditioner for subnet 2: [y1 | 1]^T
                    y_out = youts[it]
                    c2_ps = psm.tile([DIM, 128], f32, tag="tp")
                    nc.tensor.transpose(c2_ps, y_out[:, 0:DIM], identf)
                    condT = mid.tile([DIM + 1, 128], f16, tag="condT2")
                    if CONDT_ENG == "A":
                        nc.scalar.copy(condT[0:DIM, :], c2_ps)
                    else:
                        nc.vector.tensor_copy(condT[0:DIM, :], c2_ps)
                    nc.gpsimd.memset(condT[DIM:DIM + 1, :], 1.0)
                h_ps = psm.tile([51, 128], f32, tag="tp")
                nc.tensor.matmul(h_ps, w1s[s], condT, start=True, stop=True)
                hT = mid.tile([51, 128], f16, tag="hT")
                if HT_ENG == "A":
                    nc.scalar.activation(hT, h_ps, Act.Relu)
                else:
                    nc.vector.tensor_scalar(hT, h_ps, 0.0, None, Alu.max)
                big = bigp.tile([128, DIM, LS], f16, tag="big")
                minacc = tailp.tile([128, DIM], f32, tag="minacc")
                return {"hT": hT, "big": big, "minacc": minacc}

            def chunk_early(s, it, c, st):
                """PE matmuls, PSUM->SBUF copies, zm/z1/u, zn/e2/r."""
                hT, big = st["hT"], st["big"]
                xf = xfs[it]
                base = c * 3 * CHUNK
                xc32 = xf[:, s * DIM:(s + 1) * DIM]

                def copy_eng(eng, dst, src):
                    if eng == "A":
                        nc.scalar.copy(dst, src)
                    elif eng == "P":
                        nc.gpsimd.tensor_copy(dst, src)
                    else:
                        nc.vector.tensor_copy(dst, src)

                def ts_eng(eng, dst, src, s1, s2, op0, op1):
                    e = nc.gpsimd if eng == "P" else nc.vector
                    if s2 is None:
                        e.tensor_scalar(dst, src, s1, None, op0)
                    else:
                        e.tensor_scalar(dst, src, s1, s2, op0, op1)

                def tt_eng(eng, dst, a, b, op):
                    e = nc.gpsimd if eng == "P" else nc.vector
                    e.tensor_tensor(dst, a, b, op)

                # pair tiles for the late stage: z1/e2/m2s halves are
                # written per-chunk; w and t then run as single [128,1600]
                # ops per chunk-pair (amortizes the DVE per-op init).
                pc, pi = c % 2, c // 2
                if pc == 0:
                    z1P = ew.tile([128, 2, CHUNK], f16, tag="z1")
                    m2sP = ew.tile([128, 2, CHUNK], f16, tag="m2s")
                    e2P = ew.tile([128, 2, CHUNK], mybir.dt.int16, tag="e2")
                    st[("pair", pi)] = (z1P, m2sP, e2P)
                z1P, m2sP, e2P = st[("pair", pi)]

                # consumer-aligned copy order: m2s+m1s unblock u/zm after
                # two Act ops; b1s (only needed by z1) last.
                m2p = pmm.tile([128, 2, 512], f32, tag="mm")
                for hh in range(2):
                    o = base + 2 * CHUNK + hh * HALF
                    nc.tensor.matmul(m2p[:, hh, 0:HALF], hT,
                                     w2s[s][:, o:o + HALF],
                                     start=True, stop=True)
                m2s = m2sP[:, pc, :]
                m2s2 = m2s.rearrange("p (h q) -> p h q", h=2)
                copy_eng(COPIES["m2s"], m2s2, m2p[:, :, 0:HALF])

                m1p = pmm.tile([128, 2, 512], f32, tag="mm")
                for hh in range(2):
                    o = base + hh * HALF
                    nc.tensor.matmul(m1p[:, hh, 0:HALF], hT,
                                     w2s[s][:, o:o + HALF],
                                     start=True, stop=True)
                m1s = ew.tile([128, CHUNK], f16, tag="m1s")
                m1s2 = m1s.rearrange("p (h q) -> p h q", h=2)
                copy_eng(COPIES["m1s"], m1s2, m1p[:, :, 0:HALF])

                # -- z = x*m1 (per-dim broadcast), split across engines --
                # (emitted before b1s so Act's zm dims precede b1s: z1's
                # last dependency is then Act's final op of this chunk)
                zm = ew.tile([128, CHUNK], f16, tag="zm")
                zm3 = zm.rearrange("p (d l) -> p d l", l=LS)
                m1s3 = m1s.rearrange("p (d l) -> p d l", l=LS)
                for j in range(DPC):
                    xj = xc32[:, c * DPC + j:c * DPC + j + 1]
                    if j < ZM_D:
                        nc.vector.tensor_scalar_mul(
                            zm3[:, j, :], m1s3[:, j, :], xj)
                    elif j < ZM_D + ZM_A:
                        nc.scalar.activation(zm3[:, j, :], m1s3[:, j, :],
                                             Act.Copy, scale=xj)
                    else:
                        nc.gpsimd.tensor_scalar_mul(
                            zm3[:, j, :], m1s3[:, j, :], xj)

                b1p = pmm.tile([128, 2, 512], f32, tag="mm")
                for hh in range(2):
                    o = base + CHUNK + hh * HALF
                    nc.tensor.matmul(b1p[:, hh, 0:HALF], hT,
                                     w2s[s][:, o:o + HALF],
                                     start=True, stop=True)
                b1s = ew.tile([128, CHUNK], f16, tag="b1s")
                b1s2 = b1s.rearrange("p (h q) -> p h q", h=2)
                copy_eng(COPIES["b1s"], b1s2, b1p[:, :, 0:HALF])
                # z1 = zm + (b1+1) into its pair-tile half
                z1 = z1P[:, pc, :]
                z1e = Z1_ENG[c % len(Z1_ENG)] if isinstance(Z1_ENG, list) else Z1_ENG
                tt_eng(z1e, z1, zm, b1s, Alu.add)
                # v = m1*m2 (den path; needs only the first two Act copies)
                v = ew.tile([128, CHUNK], f16, tag="v")
                tt_eng(V_ENG, v, m1s, m2s, Alu.mult)
                st[("late", c)] = [z1, None, m2s, v, zm]

            def chunk_r(s, it, c, st):
                """Mid stage (slot+1): zn/e2 (exp bit-trick) and den partial
                sums: per-dim min(m1*m2, 0) accumulated into minacc via
                tensor_scalar accum_out (free); den = 1 - sum since
                sum_l relu(-v) = -sum_l min(v, 0).
                Scratch writes go into the dead zm tile regions."""
                late = st[("late", c)]
                z1, _, m2s, v, zm = late
                minacc = st["minacc"]
                # zn = min(z1, 1); e = exp(zn-1) via f16 bit-trick:
                # int16(A*zn + B) reinterpreted as f16 (Schraudolph).
                zn = ew.tile([128, CHUNK], f16, tag="zn")
                zne = ZN_ENG[c % len(ZN_ENG)] if isinstance(ZN_ENG, list) else ZN_ENG
                ts_eng_g(zne, zn, z1, 1.0, None, Alu.min, Alu.bypass)
                e2 = st[("pair", c // 2)][2][:, c % 2, :]
                ts_eng_g(E2_ENG, e2, zn, E_BIT_A, E_BIT_B, Alu.mult, Alu.add)
                late[1] = e2
                v3 = v.rearrange("p (d l) -> p d l", l=LS)
                zm3 = zm.rearrange("p (d l) -> p d l", l=LS)
                for j in range(DPC):
                    d = c * DPC + j
                    nc.vector.tensor_scalar(
                        zm3[:, j, :], v3[:, j, :], 0.0, 0.0,
                        Alu.min, Alu.add, accum_out=minacc[:, d:d + 1])

            def ts_eng_g(eng, dst, src, s1, s2, op0, op1):
                e = nc.gpsimd if eng == "P" else nc.vector
                if s2 is None:
                    e.tensor_scalar(dst, src, s1, None, op0)
                else:
                    e.tensor_scalar(dst, src, s1, s2, op0, op1)

            def chunk_late(s, it, c, st):
                """w = max(z1, e); t = w*m2 into big — pair-granular
                ([128,1600] ops, emitted at odd chunk indices)."""
                if c % 2 == 0:
                    return
                big = st["big"]
                pi = c // 2
                z1P, m2sP, e2P = st.pop(("pair", pi))
                st.pop(("late", c - 1))
                st.pop(("late", c))
                w = ew.tile([128, 2, CHUNK], f16, tag="w")
                e = nc.gpsimd if W_ENG == "P" else nc.vector
                e.tensor_tensor(w, z1P, e2P[:, :, :].bitcast(f16), Alu.max)
                tdst = big[:, 2 * pi * DPC:(2 * pi + 2) * DPC, :]
                e = nc.gpsimd if T_ENG == "P" else nc.vector
                e.tensor_tensor(tdst,
                                w.rearrange("p h (d l) -> p (h d) l", l=LS),
                                m2sP.rearrange("p h (d l) -> p (h d) l", l=LS),
                                Alu.mult)

            def fold_tail(s, it, st):
                hT, big = st["hT"], st["big"]
                xf = xfs[it]
                y_out = youts[it]
                xc32 = xf[:, s * DIM:(s + 1) * DIM]
                minacc = st["minacc"]
                # ---- fold tree over l: [128,32,100] -> [128,32] f32 ----
                n1 = big[:, :, 0:50]
                nc.vector.tensor_add(n1, big[:, :, 0:50], big[:, :, 50:100])
                n2 = big[:, :, 0:25]
                nc.vector.tensor_add(n2, n1[:, :, 0:25], n1[:, :, 25:50])
                n3 = big[:, :, 25:37]
                nc.vector.tensor_add(n3, n2[:, :, 0:12], n2[:, :, 12:24])
                n4 = big[:, :, 37:43]
                nc.vector.tensor_add(n4, n3[:, :, 0:6], n3[:, :, 6:12])
                n5 = big[:, :, 43:46]
                nc.vector.tensor_add(n5, n4[:, :, 0:3], n4[:, :, 3:6])
                s1 = big[:, :, 46]
                nc.vector.tensor_add(s1, n5[:, :, 0], n5[:, :, 1])
                s2 = big[:, :, 47]
                nc.vector.tensor_add(s2, n5[:, :, 2], n2[:, :, 24])
                numden = tailp.tile([128, DIM], f32, tag="numden")
                nc.vector.tensor_add(numden, s1, s2)

                # ---- tail (bias2 | eps | alpha | S2) ----
                tp = psm.tile([128, 4 * DIM], f32, tag="tp")
                nc.tensor.matmul(tp, hT, w2s[s][:, 3 * DL:3 * DL + 4 * DIM],
                                 start=True, stop=True)
                b2p = tp[:, 0:DIM]
                epp = tp[:, DIM:2 * DIM]
                alp = tp[:, 2 * DIM:3 * DIM]
                s2p = tp[:, 3 * DIM:4 * DIM]

                den = tailp.tile([128, DIM], f32, tag="den")
                nc.gpsimd.tensor_scalar(den, minacc, -1.0, 1.0, Alu.mult, Alu.add)
                rec = tailp.tile([128, DIM], f32, tag="rec")
                nc.vector.reciprocal_approx_fast(rec, den)
                # 0.8*sigmoid(eps/10) = 0.4*(1 + tanh(eps/20)); Tanh shares
                # the exp table-set so no ACT_TABLE_LOAD thrash.
                t2 = tailp.tile([128, DIM], f32, tag="t2")
                nc.scalar.activation(t2, epp, Act.Tanh, scale=0.05)
                ea = tailp.tile([128, DIM], f32, tag="ea")
                nc.scalar.activation(ea, alp, Act.Exp, scale=0.1)
                nums = tailp.tile([128, DIM], f32, tag="nums")
                nc.vector.tensor_sub(nums, numden, s2p)
                frac = tailp.tile([128, DIM], f32, tag="frac")
                nc.vector.scalar_tensor_tensor(
                    frac, in0=nums, scalar=0.4, in1=rec, op0=Alu.mult, op1=Alu.mult)
                q = tailp.tile([128, DIM], f32, tag="q")
                nc.vector.scalar_tensor_tensor(
                    q, in0=t2, scalar=1.0, in1=frac, op0=Alu.add, op1=Alu.mult)
                sx = tailp.tile([128, DIM], f32, tag="sx")
                e = nc.gpsimd if SX_ENG == "P" else nc.vector
                e.tensor_tensor(sx, q, xc32, Alu.add)
                yp = tailp.tile([128, DIM], f32, tag="yp")
                e = nc.gpsimd if YP_ENG == "P" else nc.vector
                e.tensor_tensor(yp, ea, sx, Alu.mult)
                nc.vector.tensor_add(y_out[:, s * DIM:(s + 1) * DIM], yp, b2p)
                if s == 1:
                    r0 = it * 128
                    nc.sync.dma_start(y_d[r0:r0 + 128, :], y_out)

            # ---- flat software-pipelined emission ----
            units = [(0, it) for it in range(NT)] + [(1, it) for it in range(NT)]
            slots = [(ui, c) for ui in range(len(units)) for c in range(NCHUNK)]
            NSL = len(slots)
            states = {}
            pending_ft = []  # unit indices whose fold/tail is not yet emitted
            for k in range(NSL + SKEW):
                if k < NSL:
                    ui, c = slots[k]
                    s, it = units[ui]
                    if c == 0 and ui not in states:
                        states[ui] = unit_head(s, it)
                    chunk_early(s, it, c, states[ui])
                    if c == HEAD_PREFETCH_AT and ui + 1 < len(units):
                        ns_, nit = units[ui + 1]
                        states[ui + 1] = unit_head(ns_, nit)
                ri = k - RSKEW
                if 0 <= ri < NSL:
                    rui, rc = slots[ri]
                    rs, rit = units[rui]
                    chunk_r(rs, rit, rc, states[rui])
                li = k - SKEW
                if 0 <= li < NSL:
                    pui, pc = slots[li]
                    ps, pit = units[pui]
                    chunk_late(ps, pit, pc, states[pui])
                    if pc == NCHUNK - 1:
                        pending_ft.append(pui)
                if pending_ft and (k >= NSL or slots[k][1] == 1):
                    fui = pending_ft.pop(0)
                    fs, fit = units[fui]
                    fold_tail(fs, fit, states[fui])
                    del states[fui]
            for fui in pending_ft:
                fs, fit = units[fui]
                fold_tail(fs, fit, states[fui])
                del states[fui]

    nc.compile()
    return nc


def _prep_weights(W1, b1, W2, b2):
    w1a = np.concatenate([W1, b1[None, :]], axis=0).astype(np.float16)  # [33, 50]
    ones_col = np.zeros((DIM + 1, 1), dtype=np.float16)
    ones_col[DIM, 0] = 1.0
    w1a = np.concatenate([w1a, ones_col], axis=1)                       # [33, 51]
    w2a = np.concatenate([W2, b2[None, :]], axis=0)                     # [51, 9696] f32
    w2a = w2a.copy()
    w2a[50, DL:2 * DL] += 1.0   # bias1 region delivers b1+1 (z1 = z+1)
    # append S2 columns: S2[:, d] = sum_l w2a[:, mat2 region (d, l)]
    m2cols = w2a[:, 2 * DL:3 * DL].reshape(51, DIM, LS)
    s2 = m2cols.sum(axis=2)                                             # [51, DIM]
    w2a = np.concatenate([w2a, s2], axis=1)                             # [51, 9728]
    # interleave chunk-blocks: [m1_c | b1_c | m2_c] x 4 chunks, tails last
    out = np.empty_like(w2a)
    for c in range(NCHUNK):
        src_m1 = w2a[:, c * CHUNK:(c + 1) * CHUNK]
        src_b1 = w2a[:, DL + c * CHUNK:DL + (c + 1) * CHUNK]
        src_m2 = w2a[:, 2 * DL + c * CHUNK:2 * DL + (c + 1) * CHUNK]
        base = c * 3 * CHUNK
        out[:, base:base + CHUNK] = src_m1
        out[:, base + CHUNK:base + 2 * CHUNK] = src_b1
        out[:, base + 2 * CHUNK:base + 3 * CHUNK] = src_m2
    out[:, 3 * DL:] = w2a[:, 3 * DL:]
    return (np.ascontiguousarray(w1a),
            np.ascontiguousarray(out.astype(np.float16)))


def kernel(**inputs):
    from concourse.bass_utils import run_bass_kernel_spmd

    if "nc" not in _cache:
        _cache["nc"] = _build_program()
    nc = _cache["nc"]

    x = np.ascontiguousarray(inputs["x"], dtype=np.float32)
    w1a1, w2a1 = _prep_weights(inputs["s1_W1"], inputs["s1_b1"],
                               inputs["s1_W2"], inputs["s1_b2"])
    w1a2, w2a2 = _prep_weights(inputs["s2_W1"], inputs["s2_b1"],
                               inputs["s2_W2"], inputs["s2_b2"])

    in_maps = []
    for i in range(NCORES):
        in_maps.append({
            "x": x[i * BC:(i + 1) * BC],
            "w1a1": w1a1, "w2a1": w2a1,
            "w1a2": w1a2, "w2a2": w2a2,
        })

    last_err = None
    for attempt in range(3):
        try:
            res = run_bass_kernel_spmd(nc, in_maps, core_ids=list(range(NCORES)),
                                       **_cache.get("run_kwargs", {}))
            out = np.concatenate([r["y"] for r in res.results], axis=0)
            _cache["last_results"] = res
            return out
        except Exception as ex:  # transient NRT/device errors: retry
            last_err = ex
    raise last_err
